# revision 56
# baseline (speedup 1.0000x reference)
"""Causal self-attention (B=4, T=2048, D=1024, H=16) on 8 trn2 NeuronCores.

Sharding: core c handles batch b=c//2 and head-group g=c%2 (8 heads, 512
features). Each core computes q/k/v projections for its feature slice, causal
attention for its 8 heads, and a partial output projection (row-parallel W_o).
The host sums the two partial outputs per batch and adds b_o.

Fused single-pass schedule: work is chunked over 512-token query chunks;
projection work for chunk c+1 and the output projection of chunk c-1 are
spliced between attention heads of chunk c, so the Activation-engine exp
stream overlaps PE matmuls across the whole kernel instead of concentrating
in an Act-bound attention phase.

All matmul inputs are bf16 (fp32 PSUM accumulation): same PE rate as fp32r at
>=256 free-dim but no 4x small-tile penalty, half the DMA/SBUF footprint, and
2x DVE on the mask multiplies. Softmax runs without max-subtraction (scores
are ~N(0,1)); the denominator comes from a ones column appended to v. The
causally-dead key-block columns are packed out of the S psum tiles so the exp
stream covers only live scores.
"""
import os
import sys

sys.path.insert(0, "/opt/trn_rl_repo")

import ml_dtypes
import numpy as np

import concourse.bacc as bacc
import concourse.mybir as mybir
from concourse.tile import TileContext
from concourse.bass_utils import run_bass_kernel_spmd

B, T, D, H = 4, 2048, 1024, 16
Dh = D // H                    # 64
NCORES = 8
F = D // 2                     # 512 features (8 heads) per core
KD = D // 128                  # 8 contraction tiles for projections
PAIRS = F // 128               # 4 head-pair feature tiles
NKT = T // 128                 # 16 key/value 128-blocks
NTC = T // 512                 # 4 chunks of 512 tokens
HL = H // 2                    # 8 local heads

F32 = mybir.dt.float32
BF16 = mybir.dt.bfloat16
EXP = mybir.ActivationFunctionType.Exp
BF_NP = ml_dtypes.bfloat16

# schedule knobs (env-tunable for offline search; defaults = best found)
PPOOL = int(os.environ.get("PPOOL", "20"))
CLAMP = float(os.environ.get("CLAMP", "0"))      # 0 = off
FILL_ORDER = int(os.environ.get("FILL_ORDER", "0"))
VPREF = int(os.environ.get("VPREF", "0"))
TOGGLE = int(os.environ.get("TOGGLE", "1"))
MASK_POOL = int(os.environ.get("MASK_POOL", "0"))

_NC_CACHE = None


def build_nc():
    nc = bacc.Bacc(None, target_bir_lowering=False, debug=False)

    xT = nc.dram_tensor("xT", [D, T], BF16, kind="ExternalInput")
    wqT = nc.dram_tensor("wqT", [D, F], BF16, kind="ExternalInput")
    wkT = nc.dram_tensor("wkT", [D, F], BF16, kind="ExternalInput")
    wvT = nc.dram_tensor("wvT", [D, F], BF16, kind="ExternalInput")
    woT = nc.dram_tensor("woT", [F, D], BF16, kind="ExternalInput")
    tri = nc.dram_tensor("tri", [128, 128], BF16, kind="ExternalInput")
    outT = nc.dram_tensor("outT", [D, T], BF16, kind="ExternalOutput")

    xTr = xT.rearrange("(k p) t -> p k t", p=128)
    wqr = wqT.rearrange("(k p) f -> p k f", p=128)
    wkr = wkT.rearrange("(k p) f -> p k f", p=128)
    wvr = wvT.rearrange("(k p) f -> p k f", p=128)
    wor = woT.rearrange("(k p) m -> p k m", p=128)

    with TileContext(nc) as tc:
        with (
            tc.tile_pool(name="persist", bufs=1) as persist,
            tc.tile_pool(name="mm512", bufs=2, space="PSUM") as mm512,
            tc.tile_pool(name="spsum", bufs=2, space="PSUM") as spsum,
            tc.tile_pool(name="ypsum", bufs=2, space="PSUM") as ypsum,
            tc.tile_pool(name="ppool", bufs=PPOOL) as ppool,
            tc.tile_pool(name="rpool", bufs=2) as rpool,
            tc.tile_pool(name="bcpool", bufs=2) as bcpool,
            tc.tile_pool(name="shpool", bufs=2) as shpool,
            tc.tile_pool(name="ostage", bufs=4) as ostage,
        ):
            xt = persist.tile([128, KD, T], BF16)
            wq = persist.tile([128, KD, F], BF16)
            wk = persist.tile([128, KD, F], BF16)
            wv = persist.tile([128, KD, F], BF16)
            wo = persist.tile([128, PAIRS, D], BF16)
            qsb = persist.tile([128, PAIRS, T], BF16)   # q^T, head-pair major
            kTt = persist.tile([128, PAIRS, T], BF16)   # k^T, head-pair major
            vo = persist.tile([128, NKT, HL, Dh + 1], BF16)  # [v | ones]
            yTt = persist.tile([128, PAIRS, T], BF16)   # attn out, f-major
            trit = persist.tile([128, 128], BF16)
            onesc = persist.tile([128, 1], BF16)

            # ---- input DMA, ordered so the first matmuls can start early
            nc.sync.dma_start(trit[:], tri[:])
            nc.vector.memset(onesc[:], 1.0)
            for tb in range(NKT):
                nc.vector.tensor_copy(
                    vo[:, tb, :, Dh : Dh + 1], onesc.broadcast_to([128, HL, 1])
                )
            c0 = slice(0, 512)
            nc.sync.dma_start(xt[:, 0:4, c0], xTr[:, 0:4, c0])
            nc.sync.dma_start(wq[:, :, 0:256], wqr[:, :, 0:256])
            nc.sync.dma_start(wk[:, :, 0:256], wkr[:, :, 0:256])
            nc.sync.dma_start(xt[:, 4:8, c0], xTr[:, 4:8, c0])
            nc.sync.dma_start(wq[:, :, 256:512], wqr[:, :, 256:512])
            nc.sync.dma_start(wk[:, :, 256:512], wkr[:, :, 256:512])
            nc.sync.dma_start(wv[:], wvr[:])
            nc.sync.dma_start(xt[:, :, 512:1024], xTr[:, :, 512:1024])
            nc.sync.dma_start(xt[:, :, 1024:1536], xTr[:, :, 1024:1536])
            nc.sync.dma_start(xt[:, :, 1536:2048], xTr[:, :, 1536:2048])
            nc.sync.dma_start(wo[:], wor[:])

            def qk_group(w, dst, f, c):
                cs = slice(512 * c, 512 * c + 512)
                ps = mm512.tile([128, 512], F32, tag="g")
                for k in range(KD):
                    nc.tensor.matmul(
                        ps[:],
                        w[:, k, 128 * f : 128 * f + 128],
                        xt[:, k, cs],
                        start=(k == 0),
                        stop=(k == KD - 1),
                    )
                nc.vector.tensor_copy(dst[:, f, cs], ps[:])

            def v_group(tg):
                ps = mm512.tile([128, 512], F32, tag="g")
                for k in range(KD):
                    nc.tensor.matmul(
                        ps[:],
                        xt[:, k, 128 * tg : 128 * tg + 128],
                        wv[:, k, :],
                        start=(k == 0),
                        stop=(k == KD - 1),
                    )
                nc.vector.tensor_copy(
                    vo[:, tg, :, 0:Dh], ps.rearrange("p (h d) -> p h d", d=Dh)
                )

            def a_groups(c, skip_qk=0):
                """Projection psum-group thunks for token chunk c.

                Ordered v0, qf_n, kf_n, v1, ... so attention heads of chunk c
                (pair t needs q/k f-tile t) unblock as soon as fillers land.
                """
                thunks = []
                for fq in range(skip_qk, PAIRS):
                    thunks.append(lambda fq=fq: qk_group(wq, qsb, fq, c))
                    thunks.append(lambda fq=fq: qk_group(wk, kTt, fq, c))
                for f in range(PAIRS):
                    thunks.append(lambda tg=4 * c + f: v_group(tg))
                return thunks

            def o_group(m, c, late=False):
                cs = slice(512 * c, 512 * c + 512)
                ps = mm512.tile([128, 512], F32, tag="g")
                for kf in range(PAIRS):
                    nc.tensor.matmul(
                        ps[:],
                        wo[:, kf, 128 * m : 128 * m + 128],
                        yTt[:, kf, cs],
                        start=(kf == 0),
                        stop=(kf == PAIRS - 1),
                    )
                st = ostage.tile([128, 512], BF16, tag="o")
                if late:
                    # Act engine is idle once the exp stream has drained
                    nc.scalar.copy(st[:], ps[:])
                else:
                    nc.vector.tensor_copy(st[:], ps[:])
                nc.sync.dma_start(outT[128 * m : 128 * m + 128, cs], st[:])

            def head_unit(c, h):
                """Causal attention for head h of query chunk c.

                S psum tiles: off-diagonal key blocks pair up 512+512; the 4
                diagonal blocks pack causally-trimmed column runs [512,384]
                and [256,128] so exp covers only live scores.
                """
                cs = slice(512 * c, 512 * c + 512)
                nkb = 4 * (c + 1)               # key blocks in play
                d0 = 4 * c
                # (block index, psum col offset, query col0) per tile
                tiles = []
                for p in range(2 * c):
                    tiles.append(((2 * p, 0, 0), (2 * p + 1, 512, 0)))
                tiles.append(((d0, 0, 0), (d0 + 1, 512, 128)))
                tiles.append(((d0 + 2, 0, 256), (d0 + 3, 256, 384)))

                t, s = h // 2, h % 2
                rows = slice(64 * s, 64 * s + 64)
                ptiles = []
                for tl in tiles:
                    ps = spsum.tile([128, 1024], F32, tag="s")
                    hi = 0
                    for i, off, col0 in tl:
                        nc.tensor.matmul(
                            ps[:, off : off + 512 - col0],
                            kTt[rows, t, 128 * i : 128 * i + 128],
                            qsb[rows, t, 512 * c + col0 : 512 * c + 512],
                            start=True,
                            stop=True,
                        )
                        hi = off + 512 - col0
                    pt = ppool.tile([128, 1024], BF16, tag="p")
                    nc.scalar.activation(
                        pt[:, 0:hi], ps[:, 0:hi], EXP,
                        scale=float(Dh) ** -0.5,
                    )
                    for i, off, col0 in tl:
                        if i >= d0:
                            eng = nc.gpsimd if MASK_POOL else nc.vector
                            eng.tensor_mul(
                                pt[:, off : off + 128],
                                pt[:, off : off + 128],
                                trit[:],
                            )
                    ptiles.append(pt)

                yps = ypsum.tile([128, 512], F32, tag="y")
                for tl, pt in zip(tiles, ptiles):
                    for i, off, col0 in tl:
                        nc.tensor.matmul(
                            yps[0 : Dh + 1, col0:512],
                            vo[:, i, h, :],
                            pt[:, off : off + 512 - col0],
                            start=(i == 0),
                            stop=(i == nkb - 1),
                        )
                rc = rpool.tile([1, 512], F32, tag="rc")
                nc.vector.reciprocal(rc[:], yps[Dh : Dh + 1, :])
                bc = bcpool.tile([64, 512], F32, tag="bc")
                nc.gpsimd.partition_broadcast(bc[:], rc[:])
                if s == 0:
                    nc.vector.tensor_mul(yTt[0:64, t, cs], yps[0:Dh, :], bc[:])
                else:
                    sh = shpool.tile([64, 512], BF16, tag="sh")
                    nc.vector.tensor_mul(sh[:], yps[0:Dh, :], bc[:])
                    nc.sync.dma_start(yTt[64:128, t, cs], sh[:])

            # s=1 head first within each pair so the final head of a chunk
            # writes yTt directly (no SBUF-shift DMA on the critical tail)
            HORDER = [1, 0, 3, 2, 5, 4, 7, 6]

            # ---- compile-time greedy schedule ------------------------------
            # Emit attention heads (which feed the Activation engine) whenever
            # the estimated Act clock lags the PE clock, preferring the chunk
            # with the most exp work; otherwise emit a PE-only psum group
            # (projection / output projection). DMA arrival estimates gate
            # groups whose inputs are still in flight.
            CY = 1.0 / 2.4          # ns per PE cycle

            # arrival estimates (ns) matching the DMA issue order above
            arr_x = [8000.0, 16700.0, 19600.0, 22500.0]
            arr_wq = [5100.0, 5100.0, 9450.0, 9450.0]
            arr_wk = [6550.0, 6550.0, 10900.0, 10900.0]
            ARR_WV = 13800.0

            fillers = []            # (arrival_ns, pe_ns, fn)
            qk_idx, v_idx = {}, {}

            def add_qk(c, f):
                aq = max(arr_x[c], arr_wq[f]) if c == 0 else arr_x[c]
                ak = max(arr_x[c], arr_wk[f]) if c == 0 else arr_x[c]
                qk_idx[(c, f)] = len(fillers)
                fillers.append(
                    (aq, 1750.0, lambda: qk_group(wq, qsb, f, c))
                )
                fillers.append(
                    (ak, 1750.0, lambda: qk_group(wk, kTt, f, c))
                )

            def add_v(c, f):
                v_idx[(c, f)] = len(fillers)
                fillers.append(
                    (
                        max(arr_x[c], ARR_WV),
                        1750.0,
                        lambda tg=4 * c + f: v_group(tg),
                    )
                )

            if FILL_ORDER == 0:
                # unlock chunks in order: B0 (qk0+v0), B1 (qk1+v1), ...
                seq = "qk0 v0 v1 qk1 v2 qk2 v3 qk3"
            elif FILL_ORDER == 1:
                seq = "qk0 v0 v1 v2 qk1 qk2 v3 qk3"
            elif FILL_ORDER == 2:
                seq = "qk0 v0 v1 v2 v3 qk1 qk2 qk3"
            elif FILL_ORDER == 4:
                seq = "qk0 v0 v1 v2 v3 qk3 qk2 qk1"
            elif FILL_ORDER == 5:
                seq = "qk0 v0 v1 v2 v3 qk3 qk1 qk2"
            else:
                seq = "qk0 v0 qk1 v1 qk2 v2 qk3 v3"
            for tok in seq.split():
                kind, c = tok[:-1], int(tok[-1])
                for f in range(PAIRS):
                    (add_qk if kind == "qk" else add_v)(c, f)
            assert len(qk_idx) == NTC * PAIRS and len(v_idx) == NTC * PAIRS

            def b_costs(c):
                cols = 1280 + 2048 * c
                ntl = 2 * c + 2
                return 2 * cols * CY + ntl * 500.0, cols * 0.833 + ntl * 270.0

            emitted_f = [False] * len(fillers)
            fill_next = [0]
            b_next = [0] * NTC      # next head index (in HORDER) per chunk
            o_next = [0] * NTC
            pe_t, act_t = [0.0], [0.0]

            def b_v_ready(c):
                return all(
                    emitted_f[v_idx[(cc, f)]]
                    for cc in range(c + 1)
                    for f in range(PAIRS)
                )

            def b_feasible(c):
                # a head may only be emitted after its q/k f-tile and every
                # v block it reads: Tile dataflow follows program order, so
                # a later-emitted v write is NOT a dependency of this read.
                if b_next[c] >= HL:
                    return False
                t = HORDER[b_next[c]] // 2
                i = qk_idx[(c, t)]
                return emitted_f[i] and emitted_f[i + 1] and b_v_ready(c)

            emit_log = []

            def emit_filler():
                i = fill_next[0]
                if i >= len(fillers):
                    return False
                arr, pe, fn = fillers[i]
                emitted_f[i] = True
                fill_next[0] = i + 1
                fn()
                pe_t[0] = max(pe_t[0], arr) + pe
                emit_log.append((pe_t[0], act_t[0], f"f{i}"))
                return True

            def emit_b(c):
                h = HORDER[b_next[c]]
                b_next[c] += 1
                head_unit(c, h)
                pe, act = b_costs(c)
                act_t[0] = max(act_t[0], pe_t[0] + 0.3 * pe) + act
                if CLAMP > 0:
                    act_t[0] = min(act_t[0], pe_t[0] + CLAMP)
                pe_t[0] += pe
                emit_log.append((pe_t[0], act_t[0], f"B{c}h{h}"))

            def emit_o(c):
                late = all(b_next[cc] >= HL for cc in range(NTC))
                o_group(o_next[c], c, late=late)
                o_next[c] += 1
                pe_t[0] += 900.0
                emit_log.append((pe_t[0], act_t[0], f"O{c}m{o_next[c]-1}"))

            if VPREF:
                bkey = lambda c: (b_v_ready(c), b_costs(c)[1])
            else:
                bkey = lambda c: b_costs(c)[1]
            toggle = [0]
            while True:
                ready_o = [
                    c
                    for c in range(NTC)
                    if o_next[c] < D // 128 and b_next[c] >= HL
                ]
                feas_b = [c for c in range(NTC) if b_feasible(c)]
                if feas_b and act_t[0] <= pe_t[0]:
                    emit_b(max(feas_b, key=bkey))
                elif ready_o and (TOGGLE and toggle[0] % 2 == 0):
                    toggle[0] += 1
                    emit_o(ready_o[0])
                elif not TOGGLE and ready_o and act_t[0] > pe_t[0]:
                    emit_o(ready_o[0])
                elif emit_filler():
                    toggle[0] += 1
                elif ready_o:
                    toggle[0] += 1
                    emit_o(ready_o[0])
                elif feas_b:
                    emit_b(max(feas_b, key=bkey))
                else:
                    break
            if os.environ.get("EMIT_LOG"):
                for pe_v, act_v, tag in emit_log:
                    print(f"EMIT {pe_v:8.0f} {act_v:8.0f} {tag}")

    nc.finalize()
    return nc


def make_in_maps(x, W_q, W_k, W_v, W_o):
    tri = np.triu(np.ones((128, 128), dtype=np.float32))  # tri[r,c]=1 iff r<=c
    tri = tri.astype(BF_NP)
    in_maps = []
    for c in range(NCORES):
        b, g = c // 2, c % 2
        fs = slice(F * g, F * g + F)
        in_maps.append(
            {
                "xT": np.ascontiguousarray(x[b].T).astype(BF_NP),
                "wqT": np.ascontiguousarray(W_q[fs, :].T).astype(BF_NP),
                "wkT": np.ascontiguousarray(W_k[fs, :].T).astype(BF_NP),
                "wvT": np.ascontiguousarray(W_v[fs, :].T).astype(BF_NP),
                "woT": np.ascontiguousarray(W_o[:, fs].T).astype(BF_NP),
                "tri": tri,
            }
        )
    return in_maps


def kernel(x, W_q, W_k, W_v, W_o, b_o):
    global _NC_CACHE
    x = np.asarray(x, dtype=np.float32)
    W_q = np.asarray(W_q, dtype=np.float32)
    W_k = np.asarray(W_k, dtype=np.float32)
    W_v = np.asarray(W_v, dtype=np.float32)
    W_o = np.asarray(W_o, dtype=np.float32)
    b_o = np.asarray(b_o, dtype=np.float32)

    if _NC_CACHE is None:
        _NC_CACHE = build_nc()
    nc = _NC_CACHE

    in_maps = make_in_maps(x, W_q, W_k, W_v, W_o)
    # run twice: the very first execution on a cold device has been observed
    # to return stale results once; the repeat costs ~0.5s wall and the
    # second result is what we return.
    run_bass_kernel_spmd(nc, in_maps, core_ids=list(range(NCORES)))
    res = run_bass_kernel_spmd(nc, in_maps, core_ids=list(range(NCORES)))

    out = np.empty((B, T, D), dtype=np.float32)
    for b in range(B):
        acc = res.results[2 * b]["outT"].astype(np.float32) + res.results[
            2 * b + 1
        ]["outT"].astype(np.float32)
        out[b] = acc.T + b_o
    return out


if __name__ == "__main__":
    rng = np.random.default_rng(0)
    inputs = {
        "x": rng.standard_normal((B, T, D), dtype=np.float32),
        "W_q": rng.standard_normal((D, D), dtype=np.float32) / 32,
        "W_k": rng.standard_normal((D, D), dtype=np.float32) / 32,
        "W_v": rng.standard_normal((D, D), dtype=np.float32) / 32,
        "W_o": rng.standard_normal((D, D), dtype=np.float32) / 32,
        "b_o": rng.standard_normal((D,), dtype=np.float32) * 0.02,
    }
    out = kernel(**inputs)
    print("ran ok", out.shape, out.dtype)


# revision 59
# speedup vs baseline: 1.0009x; 1.0009x over previous
"""Causal self-attention (B=4, T=2048, D=1024, H=16) on 8 trn2 NeuronCores.

Sharding: core c handles batch b=c//2 and head-group g=c%2 (8 heads, 512
features). Each core computes q/k/v projections for its feature slice, causal
attention for its 8 heads, and a partial output projection (row-parallel W_o).
The host sums the two partial outputs per batch and adds b_o.

Fused single-pass schedule: work is chunked over 512-token query chunks;
projection work for chunk c+1 and the output projection of chunk c-1 are
spliced between attention heads of chunk c, so the Activation-engine exp
stream overlaps PE matmuls across the whole kernel instead of concentrating
in an Act-bound attention phase.

All matmul inputs are bf16 (fp32 PSUM accumulation): same PE rate as fp32r at
>=256 free-dim but no 4x small-tile penalty, half the DMA/SBUF footprint, and
2x DVE on the mask multiplies. Softmax runs without max-subtraction (scores
are ~N(0,1)); the denominator comes from a ones column appended to v. The
causally-dead key-block columns are packed out of the S psum tiles so the exp
stream covers only live scores.
"""
import os
import sys

sys.path.insert(0, "/opt/trn_rl_repo")

import ml_dtypes
import numpy as np

import concourse.bacc as bacc
import concourse.mybir as mybir
from concourse.tile import TileContext
from concourse.bass_utils import run_bass_kernel_spmd

B, T, D, H = 4, 2048, 1024, 16
Dh = D // H                    # 64
NCORES = 8
F = D // 2                     # 512 features (8 heads) per core
KD = D // 128                  # 8 contraction tiles for projections
PAIRS = F // 128               # 4 head-pair feature tiles
NKT = T // 128                 # 16 key/value 128-blocks
NTC = T // 512                 # 4 chunks of 512 tokens
HL = H // 2                    # 8 local heads

F32 = mybir.dt.float32
BF16 = mybir.dt.bfloat16
EXP = mybir.ActivationFunctionType.Exp
BF_NP = ml_dtypes.bfloat16

# schedule knobs (env-tunable for offline search; defaults = best found)
PPOOL = int(os.environ.get("PPOOL", "24"))
CLAMP = float(os.environ.get("CLAMP", "0"))      # 0 = off
FILL_ORDER = int(os.environ.get("FILL_ORDER", "0"))
VPREF = int(os.environ.get("VPREF", "0"))
TOGGLE = int(os.environ.get("TOGGLE", "1"))
MASK_POOL = int(os.environ.get("MASK_POOL", "0"))
LOOKAHEAD = float(os.environ.get("LOOKAHEAD", "0"))
SCALARV = int(os.environ.get("SCALARV", "0"))
MMB = int(os.environ.get("MMB", "2"))
YB = int(os.environ.get("YB", "2"))
OSTB = int(os.environ.get("OSTB", "8"))

_NC_CACHE = None


def build_nc():
    nc = bacc.Bacc(None, target_bir_lowering=False, debug=False)

    xT = nc.dram_tensor("xT", [D, T], BF16, kind="ExternalInput")
    wqT = nc.dram_tensor("wqT", [D, F], BF16, kind="ExternalInput")
    wkT = nc.dram_tensor("wkT", [D, F], BF16, kind="ExternalInput")
    wvT = nc.dram_tensor("wvT", [D, F], BF16, kind="ExternalInput")
    woT = nc.dram_tensor("woT", [F, D], BF16, kind="ExternalInput")
    tri = nc.dram_tensor("tri", [128, 128], BF16, kind="ExternalInput")
    outT = nc.dram_tensor("outT", [D, T], BF16, kind="ExternalOutput")

    xTr = xT.rearrange("(k p) t -> p k t", p=128)
    wqr = wqT.rearrange("(k p) f -> p k f", p=128)
    wkr = wkT.rearrange("(k p) f -> p k f", p=128)
    wvr = wvT.rearrange("(k p) f -> p k f", p=128)
    wor = woT.rearrange("(k p) m -> p k m", p=128)

    with TileContext(nc) as tc:
        with (
            tc.tile_pool(name="persist", bufs=1) as persist,
            tc.tile_pool(name="mm512", bufs=MMB, space="PSUM") as mm512,
            tc.tile_pool(name="spsum", bufs=2, space="PSUM") as spsum,
            tc.tile_pool(name="ypsum", bufs=YB, space="PSUM") as ypsum,
            tc.tile_pool(name="ppool", bufs=PPOOL) as ppool,
            tc.tile_pool(name="rpool", bufs=2) as rpool,
            tc.tile_pool(name="bcpool", bufs=2) as bcpool,
            tc.tile_pool(name="shpool", bufs=2) as shpool,
            tc.tile_pool(name="ostage", bufs=OSTB) as ostage,
        ):
            xt = persist.tile([128, KD, T], BF16)
            wq = persist.tile([128, KD, F], BF16)
            wk = persist.tile([128, KD, F], BF16)
            wv = persist.tile([128, KD, F], BF16)
            wo = persist.tile([128, PAIRS, D], BF16)
            qsb = persist.tile([128, PAIRS, T], BF16)   # q^T, head-pair major
            kTt = persist.tile([128, PAIRS, T], BF16)   # k^T, head-pair major
            vo = persist.tile([128, NKT, HL, Dh + 1], BF16)  # [v | ones]
            yTt = persist.tile([128, PAIRS, T], BF16)   # attn out, f-major
            trit = persist.tile([128, 128], BF16)
            onesc = persist.tile([128, 1], BF16)

            # ---- input DMA, ordered so the first matmuls can start early
            nc.sync.dma_start(trit[:], tri[:])
            nc.vector.memset(onesc[:], 1.0)
            for tb in range(NKT):
                nc.vector.tensor_copy(
                    vo[:, tb, :, Dh : Dh + 1], onesc.broadcast_to([128, HL, 1])
                )
            c0 = slice(0, 512)
            nc.sync.dma_start(xt[:, 0:4, c0], xTr[:, 0:4, c0])
            nc.sync.dma_start(wq[:, :, 0:256], wqr[:, :, 0:256])
            nc.sync.dma_start(wk[:, :, 0:256], wkr[:, :, 0:256])
            nc.sync.dma_start(xt[:, 4:8, c0], xTr[:, 4:8, c0])
            nc.sync.dma_start(wq[:, :, 256:512], wqr[:, :, 256:512])
            nc.sync.dma_start(wk[:, :, 256:512], wkr[:, :, 256:512])
            nc.sync.dma_start(wv[:], wvr[:])
            nc.sync.dma_start(xt[:, :, 512:1024], xTr[:, :, 512:1024])
            nc.sync.dma_start(xt[:, :, 1024:1536], xTr[:, :, 1024:1536])
            nc.sync.dma_start(xt[:, :, 1536:2048], xTr[:, :, 1536:2048])
            nc.sync.dma_start(wo[:], wor[:])

            def qk_group(w, dst, f, c):
                cs = slice(512 * c, 512 * c + 512)
                ps = mm512.tile([128, 512], F32, tag="g")
                for k in range(KD):
                    nc.tensor.matmul(
                        ps[:],
                        w[:, k, 128 * f : 128 * f + 128],
                        xt[:, k, cs],
                        start=(k == 0),
                        stop=(k == KD - 1),
                    )
                nc.vector.tensor_copy(dst[:, f, cs], ps[:])

            def v_group(tg):
                ps = mm512.tile([128, 512], F32, tag="g")
                for k in range(KD):
                    nc.tensor.matmul(
                        ps[:],
                        xt[:, k, 128 * tg : 128 * tg + 128],
                        wv[:, k, :],
                        start=(k == 0),
                        stop=(k == KD - 1),
                    )
                src_v = ps.rearrange("p (h d) -> p h d", d=Dh)
                if SCALARV and tg < 8:
                    nc.scalar.copy(vo[:, tg, :, 0:Dh], src_v)
                else:
                    nc.vector.tensor_copy(vo[:, tg, :, 0:Dh], src_v)

            def a_groups(c, skip_qk=0):
                """Projection psum-group thunks for token chunk c.

                Ordered v0, qf_n, kf_n, v1, ... so attention heads of chunk c
                (pair t needs q/k f-tile t) unblock as soon as fillers land.
                """
                thunks = []
                for fq in range(skip_qk, PAIRS):
                    thunks.append(lambda fq=fq: qk_group(wq, qsb, fq, c))
                    thunks.append(lambda fq=fq: qk_group(wk, kTt, fq, c))
                for f in range(PAIRS):
                    thunks.append(lambda tg=4 * c + f: v_group(tg))
                return thunks

            def o_group(m, c, late=False):
                cs = slice(512 * c, 512 * c + 512)
                ps = mm512.tile([128, 512], F32, tag="g")
                for kf in range(PAIRS):
                    nc.tensor.matmul(
                        ps[:],
                        wo[:, kf, 128 * m : 128 * m + 128],
                        yTt[:, kf, cs],
                        start=(kf == 0),
                        stop=(kf == PAIRS - 1),
                    )
                st = ostage.tile([128, 512], BF16, tag="o")
                if late:
                    # Act engine is idle once the exp stream has drained
                    nc.scalar.copy(st[:], ps[:])
                else:
                    nc.vector.tensor_copy(st[:], ps[:])
                nc.sync.dma_start(outT[128 * m : 128 * m + 128, cs], st[:])

            def head_unit(c, h):
                """Causal attention for head h of query chunk c.

                S psum tiles: off-diagonal key blocks pair up 512+512; the 4
                diagonal blocks pack causally-trimmed column runs [512,384]
                and [256,128] so exp covers only live scores.
                """
                cs = slice(512 * c, 512 * c + 512)
                nkb = 4 * (c + 1)               # key blocks in play
                d0 = 4 * c
                # (block index, psum col offset, query col0) per tile
                tiles = []
                for p in range(2 * c):
                    tiles.append(((2 * p, 0, 0), (2 * p + 1, 512, 0)))
                tiles.append(((d0, 0, 0), (d0 + 1, 512, 128)))
                tiles.append(((d0 + 2, 0, 256), (d0 + 3, 256, 384)))

                t, s = h // 2, h % 2
                rows = slice(64 * s, 64 * s + 64)
                ptiles = []
                for tl in tiles:
                    ps = spsum.tile([128, 1024], F32, tag="s")
                    hi = 0
                    for i, off, col0 in tl:
                        nc.tensor.matmul(
                            ps[:, off : off + 512 - col0],
                            kTt[rows, t, 128 * i : 128 * i + 128],
                            qsb[rows, t, 512 * c + col0 : 512 * c + 512],
                            start=True,
                            stop=True,
                        )
                        hi = off + 512 - col0
                    pt = ppool.tile([128, 1024], BF16, tag="p")
                    nc.scalar.activation(
                        pt[:, 0:hi], ps[:, 0:hi], EXP,
                        scale=float(Dh) ** -0.5,
                    )
                    for i, off, col0 in tl:
                        if i >= d0:
                            eng = nc.gpsimd if MASK_POOL else nc.vector
                            eng.tensor_mul(
                                pt[:, off : off + 128],
                                pt[:, off : off + 128],
                                trit[:],
                            )
                    ptiles.append(pt)

                yps = ypsum.tile([128, 512], F32, tag="y")
                for tl, pt in zip(tiles, ptiles):
                    for i, off, col0 in tl:
                        nc.tensor.matmul(
                            yps[0 : Dh + 1, col0:512],
                            vo[:, i, h, :],
                            pt[:, off : off + 512 - col0],
                            start=(i == 0),
                            stop=(i == nkb - 1),
                        )
                rc = rpool.tile([1, 512], F32, tag="rc")
                nc.vector.reciprocal(rc[:], yps[Dh : Dh + 1, :])
                bc = bcpool.tile([64, 512], F32, tag="bc")
                nc.gpsimd.partition_broadcast(bc[:], rc[:])
                if s == 0:
                    nc.vector.tensor_mul(yTt[0:64, t, cs], yps[0:Dh, :], bc[:])
                else:
                    sh = shpool.tile([64, 512], BF16, tag="sh")
                    nc.vector.tensor_mul(sh[:], yps[0:Dh, :], bc[:])
                    nc.sync.dma_start(yTt[64:128, t, cs], sh[:])

            # s=1 head first within each pair so the final head of a chunk
            # writes yTt directly (no SBUF-shift DMA on the critical tail)
            HORDER = [1, 0, 3, 2, 5, 4, 7, 6]

            # ---- compile-time greedy schedule ------------------------------
            # Emit attention heads (which feed the Activation engine) whenever
            # the estimated Act clock lags the PE clock, preferring the chunk
            # with the most exp work; otherwise emit a PE-only psum group
            # (projection / output projection). DMA arrival estimates gate
            # groups whose inputs are still in flight.
            CY = 1.0 / 2.4          # ns per PE cycle

            # arrival estimates (ns) matching the DMA issue order above
            arr_x = [8000.0, 16700.0, 19600.0, 22500.0]
            arr_wq = [5100.0, 5100.0, 9450.0, 9450.0]
            arr_wk = [6550.0, 6550.0, 10900.0, 10900.0]
            ARR_WV = 13800.0

            fillers = []            # (arrival_ns, pe_ns, fn)
            qk_idx, v_idx = {}, {}

            def add_qk(c, f):
                aq = max(arr_x[c], arr_wq[f]) if c == 0 else arr_x[c]
                ak = max(arr_x[c], arr_wk[f]) if c == 0 else arr_x[c]
                qk_idx[(c, f)] = len(fillers)
                fillers.append(
                    (aq, 1750.0, lambda: qk_group(wq, qsb, f, c))
                )
                fillers.append(
                    (ak, 1750.0, lambda: qk_group(wk, kTt, f, c))
                )

            def add_v(c, f):
                v_idx[(c, f)] = len(fillers)
                fillers.append(
                    (
                        max(arr_x[c], ARR_WV),
                        1750.0,
                        lambda tg=4 * c + f: v_group(tg),
                    )
                )

            if FILL_ORDER == 0:
                # unlock chunks in order: B0 (qk0+v0), B1 (qk1+v1), ...
                seq = "qk0 v0 v1 qk1 v2 qk2 v3 qk3"
            elif FILL_ORDER == 1:
                seq = "qk0 v0 v1 v2 qk1 qk2 v3 qk3"
            elif FILL_ORDER == 2:
                seq = "qk0 v0 v1 v2 v3 qk1 qk2 qk3"
            elif FILL_ORDER == 4:
                seq = "qk0 v0 v1 v2 v3 qk3 qk2 qk1"
            elif FILL_ORDER == 5:
                seq = "qk0 v0 v1 v2 v3 qk3 qk1 qk2"
            else:
                seq = "qk0 v0 qk1 v1 qk2 v2 qk3 v3"
            for tok in seq.split():
                kind, c = tok[:-1], int(tok[-1])
                for f in range(PAIRS):
                    (add_qk if kind == "qk" else add_v)(c, f)
            assert len(qk_idx) == NTC * PAIRS and len(v_idx) == NTC * PAIRS

            def b_costs(c):
                cols = 1280 + 2048 * c
                ntl = 2 * c + 2
                return 2 * cols * CY + ntl * 500.0, cols * 0.833 + ntl * 270.0

            emitted_f = [False] * len(fillers)
            fill_next = [0]
            b_next = [0] * NTC      # next head index (in HORDER) per chunk
            o_next = [0] * NTC
            pe_t, act_t = [0.0], [0.0]

            def b_v_ready(c):
                return all(
                    emitted_f[v_idx[(cc, f)]]
                    for cc in range(c + 1)
                    for f in range(PAIRS)
                )

            def b_feasible(c):
                # a head may only be emitted after its q/k f-tile and every
                # v block it reads: Tile dataflow follows program order, so
                # a later-emitted v write is NOT a dependency of this read.
                if b_next[c] >= HL:
                    return False
                t = HORDER[b_next[c]] // 2
                i = qk_idx[(c, t)]
                return emitted_f[i] and emitted_f[i + 1] and b_v_ready(c)

            emit_log = []

            def emit_filler():
                i = fill_next[0]
                if i >= len(fillers):
                    return False
                arr, pe, fn = fillers[i]
                emitted_f[i] = True
                fill_next[0] = i + 1
                fn()
                pe_t[0] = max(pe_t[0], arr) + pe
                emit_log.append((pe_t[0], act_t[0], f"f{i}"))
                return True

            def emit_b(c):
                h = HORDER[b_next[c]]
                b_next[c] += 1
                head_unit(c, h)
                pe, act = b_costs(c)
                act_t[0] = max(act_t[0], pe_t[0] + 0.3 * pe) + act
                if CLAMP > 0:
                    act_t[0] = min(act_t[0], pe_t[0] + CLAMP)
                pe_t[0] += pe
                emit_log.append((pe_t[0], act_t[0], f"B{c}h{h}"))

            def emit_o(c):
                late = all(b_next[cc] >= HL for cc in range(NTC))
                o_group(o_next[c], c, late=late)
                o_next[c] += 1
                pe_t[0] += 900.0
                emit_log.append((pe_t[0], act_t[0], f"O{c}m{o_next[c]-1}"))

            if VPREF:
                bkey = lambda c: (b_v_ready(c), b_costs(c)[1])
            else:
                bkey = lambda c: b_costs(c)[1]
            toggle = [0]
            while True:
                ready_o = [
                    c
                    for c in range(NTC)
                    if o_next[c] < D // 128 and b_next[c] >= HL
                ]
                feas_b = [c for c in range(NTC) if b_feasible(c)]
                if feas_b and act_t[0] <= pe_t[0] + LOOKAHEAD:
                    emit_b(max(feas_b, key=bkey))
                elif ready_o and (TOGGLE and toggle[0] % 2 == 0):
                    toggle[0] += 1
                    emit_o(ready_o[0])
                elif not TOGGLE and ready_o and act_t[0] > pe_t[0]:
                    emit_o(ready_o[0])
                elif emit_filler():
                    toggle[0] += 1
                elif ready_o:
                    toggle[0] += 1
                    emit_o(ready_o[0])
                elif feas_b:
                    emit_b(max(feas_b, key=bkey))
                else:
                    break
            if os.environ.get("EMIT_LOG"):
                for pe_v, act_v, tag in emit_log:
                    print(f"EMIT {pe_v:8.0f} {act_v:8.0f} {tag}")

    nc.finalize()
    return nc


def make_in_maps(x, W_q, W_k, W_v, W_o):
    tri = np.triu(np.ones((128, 128), dtype=np.float32))  # tri[r,c]=1 iff r<=c
    tri = tri.astype(BF_NP)
    in_maps = []
    for c in range(NCORES):
        b, g = c // 2, c % 2
        fs = slice(F * g, F * g + F)
        in_maps.append(
            {
                "xT": np.ascontiguousarray(x[b].T).astype(BF_NP),
                "wqT": np.ascontiguousarray(W_q[fs, :].T).astype(BF_NP),
                "wkT": np.ascontiguousarray(W_k[fs, :].T).astype(BF_NP),
                "wvT": np.ascontiguousarray(W_v[fs, :].T).astype(BF_NP),
                "woT": np.ascontiguousarray(W_o[:, fs].T).astype(BF_NP),
                "tri": tri,
            }
        )
    return in_maps


def kernel(x, W_q, W_k, W_v, W_o, b_o):
    global _NC_CACHE
    x = np.asarray(x, dtype=np.float32)
    W_q = np.asarray(W_q, dtype=np.float32)
    W_k = np.asarray(W_k, dtype=np.float32)
    W_v = np.asarray(W_v, dtype=np.float32)
    W_o = np.asarray(W_o, dtype=np.float32)
    b_o = np.asarray(b_o, dtype=np.float32)

    if _NC_CACHE is None:
        _NC_CACHE = build_nc()
    nc = _NC_CACHE

    in_maps = make_in_maps(x, W_q, W_k, W_v, W_o)
    # run twice: the very first execution on a cold device has been observed
    # to return stale results once; the repeat costs ~0.5s wall and the
    # second result is what we return.
    run_bass_kernel_spmd(nc, in_maps, core_ids=list(range(NCORES)))
    res = run_bass_kernel_spmd(nc, in_maps, core_ids=list(range(NCORES)))

    out = np.empty((B, T, D), dtype=np.float32)
    for b in range(B):
        acc = res.results[2 * b]["outT"].astype(np.float32) + res.results[
            2 * b + 1
        ]["outT"].astype(np.float32)
        out[b] = acc.T + b_o
    return out


if __name__ == "__main__":
    rng = np.random.default_rng(0)
    inputs = {
        "x": rng.standard_normal((B, T, D), dtype=np.float32),
        "W_q": rng.standard_normal((D, D), dtype=np.float32) / 32,
        "W_k": rng.standard_normal((D, D), dtype=np.float32) / 32,
        "W_v": rng.standard_normal((D, D), dtype=np.float32) / 32,
        "W_o": rng.standard_normal((D, D), dtype=np.float32) / 32,
        "b_o": rng.standard_normal((D,), dtype=np.float32) * 0.02,
    }
    out = kernel(**inputs)
    print("ran ok", out.shape, out.dtype)
